# revision 30
# baseline (speedup 1.0000x reference)
"""GNN message-passing (3x GraphConv + mean-pool + classifier) on 8 Trainium2 cores.

v2: quartered, pipelined AllGather (Shared-output) + transposed aggregation.
  - Nodes dst-sharded 8 ways (12500/core, padded to 12800 = 100 blocks of 128).
  - Per-core shard split into 4 quarters of 3200 rows. Each layer's node-major
    fp16 table is published quarter-by-quarter: AllGather(quarter) fires as
    soon as the 25 blocks of that quarter are transformed, overlapping the
    collective with the remaining compute. Gather "pages" = quarters
    (25600 rows each, int16-indexable).
  - Aggregation for layers 0/1 runs feature-major (out = ge^T... via
    matmul(lhsT=ge, rhs=onehot)), so the fused next-layer transform needs no
    PE transpose / DVE copy: matmul(lhsT=ynmT, rhs=W). Layer 2 aggregates
    node-major for the pooling matmul.
  - Per-node norm scales folded into Act-engine activation (Relu / Copy with
    per-partition scale), keeping DVE exclusively for one-hot builds.
"""
import sys
import numpy as np

sys.path.insert(0, "/opt/trn_rl_repo")

import concourse.bass as bass  # noqa: E402
import concourse.bacc as bacc  # noqa: E402
import concourse.mybir as mybir  # noqa: E402
import concourse.tile as tile  # noqa: E402
from concourse.bass_utils import run_bass_kernel_spmd  # noqa: E402

# problem constants
N_NODES = 100000
N_EDGES = 1600000
N_GRAPHS = 1000
IN_DIM, HID, N_CLS = 95, 128, 16

NCORES = 8
B = 12500                        # real nodes per core
BP = 12800                       # padded rows per core (100 blocks)
NBLK = BP // 128                 # 100 blocks
NQ = 4                           # quarters per core
QR = BP // NQ                    # 3200 rows per quarter
QBLK = NBLK // NQ                # 25 blocks per quarter
PAGE = NCORES * QR               # 25600 rows per gather page (one quarter, all cores)
WINB = 4                         # blocks per window
NWIN = NBLK // WINB              # 25 windows
BUDGET = 5                       # chunks (of 128 idx) per (block x page) cell
CELL = BUDGET * 128              # 640 idx slots per cell
CALL = WINB * CELL               # 2560 idxs per dma_gather call
CCOL = CALL // 16                # 160 int16 cols per call in idx stream
NCALL = NWIN * NQ                # 100 calls per layer
CHPC = WINB * BUDGET             # 20 chunks per call
NCHUNK = NCALL * CHPC            # 2000 chunks per layer
GSPAN = 192                      # padded per-core graph span for pooling

F16 = mybir.dt.float16
F32 = mybir.dt.float32
F8 = mybir.dt.float8e4
I16 = mybir.dt.int16

_COMPILED = None


def _dma_gather_128(eng, out_ap, in_ap, idxs_ap, num_idxs, num_idxs_reg,
                    elem_size, elem_step, queue_num):
    """Vendored non-transpose DRAM-source dma_gather with the elem-size
    alignment relaxed from 256B to 128B (the 256B requirement is a
    transpose-path restriction in the ucode decode; non-transpose packets
    carry one descriptor per index of elem_size bytes)."""
    from concourse.bass import MemorySpace
    from concourse import ap_utils
    from concourse._compat import exact_div, round_up_to_multiple

    eng._assert_queue_num(queue_num)
    assert idxs_ap.dtype == mybir.dt.int16
    assert in_ap.dtype == out_ap.dtype
    elem_size_bytes = elem_size * mybir.dt.size(in_ap.dtype)
    assert elem_size_bytes > 0 and elem_size_bytes % 128 == 0
    assert in_ap.space == MemorySpace.DRAM
    assert idxs_ap.space == MemorySpace.SBUF
    assert out_ap.space == MemorySpace.SBUF
    assert ap_utils.ap_is_contiguous(out_ap.ap[1:])
    assert ap_utils.ap_is_contiguous(idxs_ap.ap[1:])
    assert in_ap.ap[-1][1] == out_ap.ap[-1][1] == elem_size
    assert out_ap.ap[0][1] * out_ap.ap[1][1] == round_up_to_multiple(
        num_idxs, 128)
    assert in_ap.ap[0][0] == elem_step
    stride_bytes = elem_step * mybir.dt.size(in_ap.dtype)
    stride_bytes_256 = exact_div(stride_bytes, 256)
    assert stride_bytes_256 < 256
    _in_ap = eng.lower_ap_dma(in_ap, for_custom_bir_dma=True)
    inst = eng.add_instruction(
        mybir.InstDMAGatherAnt(
            name=eng.bass.get_next_instruction_name(),
            ins=[
                *_in_ap,
                eng.lower_ap(idxs_ap),
                eng.lower_val_access(eng.to_reg(num_idxs_reg)),
            ],
            outs=[eng.lower_ap(out_ap)],
            transpose=False,
            num_idxs=num_idxs,
            elem_size=elem_size,
            stride_bytes_256=stride_bytes_256,
            gen_mode=0,
            single_packet=False,
            queue_num=queue_num,
        )
    )
    return inst


HB = B // 2  # real nodes per half under balanced placement (6250)


def _balance_core(prof, cap):
    """Half-constrained greedy placement + repair: local nodes [0,HB) fill
    half-0 positions, [HB,B) fill half-1, permuted so every per-(block,page)
    in-edge count <= cap.

    prof: [B, NQ] per-node in-degree by source page (computed with the same
    half split). Returns pos[B]: table position for each local node."""
    pos = np.zeros(B, np.int64)
    nhb = NBLK // 2
    for h, nodes in ((0, np.arange(0, HB)), (1, np.arange(HB, B))):
        order = nodes[np.argsort(-prof[nodes].max(1), kind="stable")]
        cell = np.zeros((nhb, NQ), np.int64)
        cnt = np.zeros(nhb, np.int64)
        assign = [[] for _ in range(nhb)]
        for v in order:
            proj = np.max(cell + prof[v][None, :], axis=1)
            proj = proj + cnt * 1e-3
            proj[cnt >= 128] = 1 << 30
            b = int(np.argmin(proj))
            assign[b].append(v)
            cell[b] += prof[v]
            cnt[b] += 1
        # repair: relocate heavy contributors out of over-cap cells
        for _ in range(20000):
            bs, ps = np.unravel_index(np.argmax(cell), cell.shape)
            worst = cell[bs, ps]
            if worst <= cap:
                break
            best = None
            for v in sorted(assign[bs], key=lambda u: -prof[u][ps])[:12]:
                if prof[v][ps] == 0:
                    break
                proj = np.max(cell + prof[v][None, :], axis=1)
                proj[cnt >= 128] = 1 << 30
                proj[bs] = 1 << 30
                b2 = int(np.argmin(proj))
                if proj[b2] < worst:
                    best = (v, b2)
                    break
            if best is None:
                break
            v, b2 = best
            assign[bs].remove(v)
            assign[b2].append(v)
            cell[bs] -= prof[v]
            cell[b2] += prof[v]
            cnt[bs] -= 1
            cnt[b2] += 1
        for b in range(nhb):
            for i, v in enumerate(assign[b]):
                pos[v] = (h * nhb + b) * 128 + i
    return pos


def _host_prep(x, src, dst, graph_id, W1, b1, W2, b2, W3, b3, Wc, bc,
               fullag=False, hag=False, budget=BUDGET, bal=False):
    """Build all per-core input streams. Index math only (+ dtype marshaling)."""
    CELL = budget * 128
    CALL = WINB * CELL
    CCOL = CALL // 16
    CHPC = WINB * budget
    src = np.asarray(src).astype(np.int64)
    dst = np.asarray(dst).astype(np.int64)
    graph_id = np.asarray(graph_id).astype(np.int64)
    x = np.asarray(x, dtype=np.float32)
    assert np.all(np.asarray(b1) == 0) and np.all(np.asarray(b2) == 0) and np.all(
        np.asarray(b3) == 0
    ), "kernel assumes zero conv biases (relu/scale folding)"

    deg_out = np.clip(np.bincount(src, minlength=N_NODES), 1, None).astype(np.float64)
    deg_in = np.clip(np.bincount(dst, minlength=N_NODES), 1, None).astype(np.float64)
    ns = (deg_out ** -0.5).astype(np.float32)
    nd = (deg_in ** -0.5).astype(np.float32)
    cnt = np.clip(np.bincount(graph_id, minlength=N_GRAPHS), 1, None).astype(np.float32)

    s_oc = src // B
    s_r = src % B
    # page of a source depends only on its quarter/half, which balancing
    # preserves, so compute pages from the identity layout first
    if fullag:
        s_page = s_oc // 2
    elif hag:
        s_page = (s_r // (2 * QR)) * 2 + s_oc // 4
    else:
        s_page = s_r // QR

    if bal:
        assert hag, "balanced placement only wired for hag page layout"
        # planned half split (HB real nodes per half) fixes every source's
        # page up front; balancing then permutes within halves only
        s_page_b = ((s_r >= HB).astype(np.int64)) * 2 + s_oc // 4
        pos_all = np.zeros(N_NODES, np.int64)
        prof_all = np.zeros((N_NODES, NQ), np.int64)
        np.add.at(prof_all, (dst, s_page_b), 1)
        for c in range(NCORES):
            gl = slice(c * B, (c + 1) * B)
            pos_all[gl] = _balance_core(prof_all[gl], CELL) + c * BP
    else:
        ids = np.arange(N_NODES, dtype=np.int64)
        pos_all = (ids // B) * BP + (ids % B)

    p_src = pos_all[src] % BP  # per-edge: source position within owner core
    if fullag:
        s_row = (s_oc % 2) * BP + p_src
    elif hag:
        s_h = p_src // (2 * QR)
        s_row = (s_oc % 4) * (2 * QR) + (p_src - s_h * 2 * QR)
        s_page = s_h * 2 + s_oc // 4
    else:
        s_row = s_oc * QR + (p_src % QR)

    core_of = dst // B
    per_core = []
    for c in range(NCORES):
        m = core_of == c
        ed = pos_all[dst[m]] - c * BP
        blk = ed >> 7
        page = s_page[m]
        lrow = s_row[m]
        slot = (ed & 127).astype(np.int64)

        idx_stream = np.zeros((NCALL, CALL), np.int64)
        slot_stream = np.full((NCALL, CALL), -1.0, np.float32)

        # bucket edges by (block, page); sort by gather row for locality
        order = np.lexsort((lrow, page, blk))
        blk_o, page_o, lrow_o, slot_o = (
            blk[order], page[order], lrow[order], slot[order])
        cell_key = blk_o * NQ + page_o
        starts = np.searchsorted(cell_key, np.arange(NBLK * NQ + 1))
        for b in range(NBLK):
            w, cw_ = divmod(b, WINB)
            for q in range(NQ):
                lo, hi = starts[b * NQ + q], starts[b * NQ + q + 1]
                n = hi - lo
                assert n <= CELL, f"cell overflow core {c} blk {b} page {q}: {n}"
                call_id = w * NQ + q
                base = cw_ * CELL
                idx_stream[call_id, base:base + n] = lrow_o[lo:hi]
                slot_stream[call_id, base:base + n] = slot_o[lo:hi]
                if n < CELL:  # mid-call pads: repeat a valid row (slot=-1)
                    fill = lrow_o[hi - 1] if n > 0 else 0
                    idx_stream[call_id, base + n:base + CELL] = fill

        # wrap idxs: flat position i -> [i%16, i//16], replicated to 128 partitions
        idx16 = idx_stream.reshape(NCALL, CCOL, 16).transpose(0, 2, 1)
        idx128 = np.tile(idx16, (1, 8, 1)).transpose(1, 0, 2).reshape(128, NCALL * CCOL)
        slot128 = slot_stream.reshape(NCALL * CHPC, 128).T.copy()

        # node -> position scatter for this core
        pos_c = pos_all[c * B:(c + 1) * B] - c * BP

        # pooling stream [128, NBLK*GSPAN]; padded rows contribute zero
        gid_c = graph_id[c * B:(c + 1) * B]
        g0 = int(gid_c.min())
        span = int(gid_c.max()) - g0 + 1
        assert span <= GSPAN, f"graph span {span} > {GSPAN}"
        pstream = np.zeros((128, NBLK * GSPAN), np.float32)
        pstream[pos_c & 127, (pos_c >> 7) * GSPAN + (gid_c - g0)] = 1.0 / cnt[gid_c]

        # per-node scalar streams [128, NBLK], zero in padded region
        def colify(v):
            flat = np.zeros(BP, np.float32)
            flat[pos_c] = v[c * B:(c + 1) * B]
            return np.ascontiguousarray(flat.reshape(NBLK, 128).T)

        xp = np.zeros((BP, IN_DIM), np.float32)
        xp[pos_c] = x[c * B:(c + 1) * B]
        xT = np.ascontiguousarray(xp.T).astype(np.float16)  # [95, BP]

        per_core.append({
            "xT": xT,
            "idxs": idx128.astype(np.int16),
            "slots": slot128.astype(np.float16),
            "slots32": slot128.astype(np.float32),
            "sc1": colify(ns),
            "sc12": colify(ns * nd),
            "sc3": colify(nd),
            "pstr": pstream.astype(np.float16),
            "g0": g0, "span": span,
        })

    # loc0 streams: full padded x^T and full ns (block-column layout), same on
    # every core
    xpf = np.zeros((NCORES * BP, IN_DIM), np.float32)
    nsf = np.zeros(NCORES * BP, np.float32)
    for c in range(NCORES):
        xpf[c * BP:c * BP + B] = x[c * B:(c + 1) * B]
        nsf[c * BP:c * BP + B] = ns[c * B:(c + 1) * B]
    xTf = np.ascontiguousarray(xpf.T).astype(np.float16)  # [95, 8*BP]
    sc1f = np.ascontiguousarray(nsf.reshape(NCORES * NBLK, 128).T).astype(np.float32)

    iota = np.ascontiguousarray(np.broadcast_to(
        np.arange(128, dtype=np.float16)[None, None, :], (128, CHPC, 128)))
    shared = {
        "xTf": xTf,
        "sc1f": sc1f,
        "iota8": iota.reshape(128, CHPC * 128),
        "W1f": np.asarray(W1, np.float32).astype(np.float16),
        "W2f": np.asarray(W2, np.float32).astype(np.float16),
        "W3f": np.asarray(W3, np.float32).astype(np.float16),
        "Wcf": np.asarray(Wc, np.float32).astype(np.float16),
    }
    return per_core, shared, cnt


def _build_nc(variant=frozenset(), reps=1, budget=BUDGET):
    variant = frozenset(variant)
    CELL = budget * 128
    CALL = WINB * CELL
    CCOL = CALL // 16
    CHPC = WINB * budget
    NCHUNK = NCALL * CHPC
    BUDGET = budget  # noqa: F841 — shadow module constant for loops below
    nqueues = 8 if "q8" in variant else 4
    nc = bacc.Bacc("TRN2", target_bir_lowering=False, debug=False,
                   num_devices=NCORES, num_swdge_queues=nqueues)
    loc0 = "loc0" in variant
    if loc0:
        xTf = nc.dram_tensor("xTf", [IN_DIM, NCORES * BP], F16, kind="ExternalInput")
        sc1f = nc.dram_tensor("sc1f", [128, NCORES * NBLK], F32, kind="ExternalInput")
    else:
        xT = nc.dram_tensor("xT", [IN_DIM, BP], F16, kind="ExternalInput")
    idxs = nc.dram_tensor("idxs", [128, NCALL * CCOL], I16, kind="ExternalInput")
    slots = None if "tshot" in variant else nc.dram_tensor(
        "slots", [128, NCHUNK], F16, kind="ExternalInput")
    slots32 = nc.dram_tensor("slots32", [128, NCHUNK], F32, kind="ExternalInput") \
        if "tshot" in variant else None
    iota8 = nc.dram_tensor("iota8", [128, CHPC * 128], F16, kind="ExternalInput")
    W1f = nc.dram_tensor("W1f", [IN_DIM, HID], F16, kind="ExternalInput")
    W2f = nc.dram_tensor("W2f", [HID, HID], F16, kind="ExternalInput")
    W3f = nc.dram_tensor("W3f", [HID, HID], F16, kind="ExternalInput")
    Wcf = nc.dram_tensor("Wcf", [HID, N_CLS], F16, kind="ExternalInput")
    if not loc0:
        sc1 = nc.dram_tensor("sc1", [128, NBLK], F32, kind="ExternalInput")
    sc12 = nc.dram_tensor("sc12", [128, NBLK], F32, kind="ExternalInput")
    sc3 = nc.dram_tensor("sc3", [128, NBLK], F32, kind="ExternalInput")
    pstr = nc.dram_tensor("pstr", [128, NBLK * GSPAN], F16, kind="ExternalInput")
    zc = nc.dram_tensor("zc", [N_CLS, GSPAN], F32, kind="ExternalOutput")

    RELU = mybir.ActivationFunctionType.Relu
    COPY = mybir.ActivationFunctionType.Copy

    with tile.TileContext(nc) as tc:
        with (
            tc.tile_pool(name="res", bufs=1) as res,
            tc.tile_pool(name="dram", bufs=1, space="DRAM") as dram,
            tc.tile_pool(name="gew", bufs=(6 if "fat" in variant else
                                           12 if "cw14" not in variant else 10)) as gew,
            tc.tile_pool(name="cw", bufs=(15 if "cwx" in variant else
                                          12 if "cw14" not in variant else 14)) as cw,
            tc.tile_pool(name="eps", bufs=6) as eps,
            tc.tile_pool(name="accp", bufs=1) as accp,
            tc.tile_pool(name="pwin", bufs=2) as pwin,
            tc.tile_pool(name="xsp", bufs=3) as xsp,
            tc.tile_pool(name="spsum", bufs=2, space="PSUM") as spsum,
            tc.tile_pool(name="tpsum", bufs=2, space="PSUM") as tpsum,
            tc.tile_pool(name="gpsum", bufs=1, space="PSUM") as gpsum,
        ):
            # resident loads
            idx_t = res.tile([128, NCALL * CCOL], I16)
            slot_t = None if "tshot" in variant else res.tile(
                [128, NCHUNK], F16, name="slot_t")
            slot32_t = res.tile([128, NCHUNK], F32, name="slot32_t") \
                if "tshot" in variant else None
            iota_t = res.tile([128, CHPC, 128], F16)
            xT_t = None if (loc0 or "cwx" in variant) else res.tile(
                [IN_DIM, BP], F16, name="xT_t")
            w1_t = res.tile([IN_DIM, HID], F16)
            w2_t = res.tile([HID, HID], F16)
            w3_t = res.tile([HID, HID], F16)
            wc_t = res.tile([HID, N_CLS], F16)
            sc1_t = None if loc0 else res.tile([128, NBLK], F32, name="sc1_t")
            sc1f_t = res.tile([128, NCORES * NBLK], F32, name="sc1f_t") if loc0 else None
            sc12_t = res.tile([128, NBLK], F32)
            sc3_t = res.tile([128, NBLK], F32)
            nc.sync.dma_start(idx_t[:], idxs[:])
            if slot_t is not None:
                nc.sync.dma_start(slot_t[:], slots[:])
            if slot32_t is not None:
                nc.sync.dma_start(slot32_t[:], slots32[:])
            nc.sync.dma_start(iota_t[:], iota8[:].rearrange("p (a b) -> p a b", a=CHPC))
            if loc0:
                nc.sync.dma_start(sc1f_t[:], sc1f[:])
            elif xT_t is not None:
                nc.sync.dma_start(xT_t[:], xT[:])
            nc.sync.dma_start(w1_t[:], W1f[:])
            nc.sync.dma_start(w2_t[:], W2f[:])
            nc.sync.dma_start(w3_t[:], W3f[:])
            nc.sync.dma_start(wc_t[:], Wcf[:])
            if not loc0:
                nc.sync.dma_start(sc1_t[:], sc1[:])
            nc.sync.dma_start(sc12_t[:], sc12[:])
            nc.sync.dma_start(sc3_t[:], sc3[:])

            yaddr = "Shared" if "shared" in variant else "Local"
            ybq = [[dram.tile([QR, HID], F16, name=f"yb{i}_{q}", tag=f"yb{i}_{q}")
                    for q in range(NQ)] for i in range(3)]

            def publish(yfp, layer, q):
                if "nocc" in variant:
                    return
                nc.gpsimd.collective_compute(
                    "AllGather", mybir.AluOpType.bypass,
                    replica_groups=[list(range(NCORES))],
                    ins=[ybq[layer][q][:].opt()], outs=[yfp[layer][q].opt()])

            def body(rep):
                ctd = (dram.tile([128, NCHUNK * 128], F16, name=f"ctd{rep}",
                                 tag=f"ctd{rep}")
                       if "ctcache" in variant else None)
                ctd2 = ([dram.tile([128, CHPC * 128], F16, name=f"ctd2_{rep}_{i}",
                                   tag=f"ctd2_{rep}_{i}") for i in range(NCALL)]
                        if "ctc2" in variant else None)
                f8 = "f8" in variant
                tdt = F8 if f8 else F16
                tw = 2 * HID if f8 else HID  # f8 rows padded to 256B stride
                if "hag" in variant:
                    ybh = [[dram.tile([2 * QR, tw], tdt, name=f"ybh{rep}_{i}_{h}",
                                      tag=f"ybh{rep}_{i}_{h}") for h in range(2)]
                           for i in range(3)]
                    yfh = [[dram.tile([2 * PAGE, tw], tdt, name=f"yfh{rep}_{i}_{h}",
                                      tag=f"yfh{rep}_{i}_{h}", addr_space=yaddr)
                            for h in range(2)] for i in range(3)]
                    yfp = [[yfh[i][q // 2][(q % 2) * PAGE:(q % 2 + 1) * PAGE, :]
                            for q in range(NQ)] for i in range(3)]
                elif "fullag" in variant:
                    ybf = [dram.tile([BP, HID], F16, name=f"ybf{rep}_{i}",
                                     tag=f"ybf{rep}_{i}") for i in range(3)]
                    yff = [dram.tile([NCORES * BP, HID], F16, name=f"yff{rep}_{i}",
                                     tag=f"yff{rep}_{i}", addr_space=yaddr)
                        for i in range(3)]
                    yfp = [[yff[i][q * PAGE:(q + 1) * PAGE, :] for q in range(NQ)]
                           for i in range(3)]
                else:
                    yfp = [[dram.tile([PAGE, HID], F16, name=f"yf{rep}_{i}_{q}",
                                      tag=f"yf{rep}_{i}_{q}",
                                      addr_space=("Local" if (loc0 and i == 0)
                                                  else yaddr))[:]
                            for q in range(NQ)] for i in range(3)]

                def emit_table_row(layer, b, tnm):
                    if "hag" in variant:
                        hh, bh = divmod(b, 2 * QBLK)
                        nc.sync.dma_start(
                            ybh[layer][hh][bh * 128:(bh + 1) * 128, :HID], tnm[:])
                        if bh == 2 * QBLK - 1 and "nocc" not in variant:
                            nc.gpsimd.collective_compute(
                                "AllGather", mybir.AluOpType.bypass,
                                replica_groups=[list(range(NCORES))],
                                ins=[ybh[layer][hh][:].opt()],
                                outs=[yfh[layer][hh][:].opt()])
                        return
                    if "fullag" in variant:
                        nc.sync.dma_start(ybf[layer][b * 128:(b + 1) * 128, :], tnm[:])
                        if b == NBLK - 1 and "nocc" not in variant:
                            nc.gpsimd.collective_compute(
                                "AllGather", mybir.AluOpType.bypass,
                                replica_groups=[list(range(NCORES))],
                                ins=[ybf[layer][:].opt()], outs=[yff[layer][:].opt()])
                    else:
                        qq, bq = divmod(b, QBLK)
                        nc.sync.dma_start(
                            ybq[layer][qq][bq * 128:(bq + 1) * 128, :], tnm[:])
                        if bq == QBLK - 1:
                            publish(yfp, layer, qq)
                # ---- phase T1: table0 = (x @ W1) * ns ----
                if loc0:
                    # every core computes the FULL table0 from replicated x:
                    # no layer-0 collectives; gathers then read local HBM
                    for gbase in range(0, NCORES * NBLK, 8):
                        xs = xsp.tile([IN_DIM, 8, 128], F16, name="xs", tag="xs")
                        nc.sync.dma_start(
                            xs[:], xTf[:, gbase * 128:(gbase + 8) * 128]
                            .rearrange("p (a b) -> p a b", a=8))
                        for k in range(8):
                            gb = gbase + k
                            oc, b = divmod(gb, NBLK)
                            qq, bq = divmod(b, QBLK)
                            tps = tpsum.tile([128, HID], F32, name="tps")
                            nc.tensor.matmul(tps[:], xs[:, k, :], w1_t[:],
                                             start=True, stop=True)
                            tnm = eps.tile([128, HID], tdt, name="tnm", tag="tnm")
                            nc.scalar.activation(tnm[:], tps[:], COPY,
                                                 scale=sc1f_t[:, gb:gb + 1])
                            row = oc * QR + bq * 128
                            nc.sync.dma_start(yfp[0][qq][row:row + 128, :], tnm[:])
                elif "cwx" in variant:
                    for bbase in range(0, NBLK, 4):
                        xs = xsp.tile([IN_DIM, 4, 128], F16, name="xs", tag="xs")
                        nc.sync.dma_start(
                            xs[:], xT[:, bbase * 128:(bbase + 4) * 128]
                            .rearrange("p (a b) -> p a b", a=4))
                        for k in range(4):
                            b = bbase + k
                            tps = tpsum.tile([128, HID], F32, name="tps")
                            nc.tensor.matmul(tps[:], xs[:, k, :], w1_t[:],
                                             start=True, stop=True)
                            tnm = eps.tile([128, HID], tdt, name="tnm", tag="tnm")
                            nc.scalar.activation(tnm[:], tps[:], COPY,
                                                 scale=sc1_t[:, b:b + 1])
                            emit_table_row(0, b, tnm)
                else:
                    for b in range(NBLK):
                        tps = tpsum.tile([128, HID], F32, name="tps")
                        nc.tensor.matmul(tps[:], xT_t[:, b * 128:(b + 1) * 128],
                                         w1_t[:], start=True, stop=True)
                        tnm = eps.tile([128, HID], tdt, name="tnm", tag="tnm")
                        nc.scalar.activation(tnm[:], tps[:], COPY,
                                             scale=sc1_t[:, b:b + 1])
                        emit_table_row(0, b, tnm)

                gacc = gpsum.tile([128, GSPAN], F32, name="gacc")

                def fetch(layer, w, q):
                    """gather + one-hot build for call (w, q)."""
                    call_id = w * NQ + q
                    qno = call_id % 8 if "q8" in variant else q
                    fat = "fat" in variant
                    ge = gew.tile([128, CHPC, 2 * HID if fat else HID],
                                  tdt, name="ge", tag="ge")
                    if "nogather" in variant:
                        nc.sync.dma_start(
                            ge[:], yfp[layer][q][:128 * CHPC, :HID]
                            .rearrange("(p a) h -> p a h", p=128))
                    elif f8:
                        _dma_gather_128(
                            nc.gpsimd, out_ap=ge[:],
                            in_ap=yfp[layer][q][:, :HID],
                            idxs_ap=idx_t[:, call_id * CCOL:(call_id + 1) * CCOL],
                            num_idxs=CALL, num_idxs_reg=CALL,
                            elem_size=HID, elem_step=2 * HID,
                            queue_num=qno)
                    else:
                        in_ap = (yfp[layer][q]
                                 .rearrange("(a b) h -> a (b h)", b=2)
                                 if fat else yfp[layer][q])
                        nc.gpsimd.dma_gather(
                            out_ap=ge[:],
                            in_ap=in_ap,
                            idxs_ap=idx_t[:, call_id * CCOL:(call_id + 1) * CCOL],
                            num_idxs=CALL, num_idxs_reg=CALL,
                            elem_size=2 * HID if fat else HID,
                            single_packet=("sp" in variant), queue_num=qno)
                    ct = cw.tile([128, CHPC, 128],
                                 F8 if "ct8" in variant else F16,
                                 name="ct", tag="ct")
                    cbase = call_id * CHPC
                    if "noonehot" in variant:
                        nc.vector.tensor_copy(ct[:], iota_t[:])
                    elif "tshot" in variant:
                        for ch in range(CHPC):
                            nc.vector.tensor_scalar(
                                out=ct[:, ch, :], in0=iota_t[:, 0, :],
                                scalar1=slot32_t[:, cbase + ch:cbase + ch + 1],
                                scalar2=None,
                                op0=mybir.AluOpType.is_equal)
                    else:
                        nc.vector.tensor_tensor(
                            out=ct[:],
                            in0=slot_t[:, cbase:cbase + CHPC, None]
                                .to_broadcast([128, CHPC, 128]),
                            in1=iota_t[:],
                            op=mybir.AluOpType.is_equal)
                    return ge, ct

                def transform_emit(layer, b, src_ap):
                    """L0/L1: relu -> W-transform -> scaled emit; L2: pool."""
                    wnext = [w2_t, w3_t, None][layer]
                    scale_t = sc12_t if layer < 2 else sc3_t
                    if layer < 2:
                        ynmT = eps.tile([128, HID], F16, name="ynmT", tag="ynmT")
                        nc.scalar.activation(ynmT[:], src_ap, RELU)
                        tps2 = tpsum.tile([128, HID], F32, name="tps2")
                        nc.tensor.matmul(tps2[:], ynmT[:], wnext[:],
                                         start=True, stop=True)
                        tnm = eps.tile([128, HID], tdt, name="tnm", tag="tnm")
                        nc.scalar.activation(tnm[:], tps2[:], COPY,
                                             scale=scale_t[:, b:b + 1])
                        emit_table_row(layer + 1, b, tnm)
                    else:
                        ynm = eps.tile([128, HID], F16, name="ynm", tag="ynm")
                        nc.scalar.activation(ynm[:], src_ap, RELU,
                                             scale=scale_t[:, b:b + 1])
                        nc.tensor.matmul(
                            gacc[:], ynm[:], pw_t[b // WINB][:, b % WINB, :],
                            start=(b == 0), stop=(b == NBLK - 1))

                pw_t = {}

                def load_pw(w):
                    pw = pwin.tile([128, WINB, GSPAN], F16, name="pw")
                    nc.sync.dma_start(
                        pw[:],
                        pstr[:, w * WINB * GSPAN:(w + 1) * WINB * GSPAN]
                        .rearrange("p (a g) -> p a g", a=WINB))
                    pw_t[w] = pw

                # ---- layers ----
                if "pm" in variant:
                    # page-major: sweep pages in publish order, accumulate in
                    # an SBUF fp32 table; collectives hide under the stream
                    acc = accp.tile([128, NBLK * 128], F16, name="acc")
                    for layer in range(3):
                        for q in range(NQ):
                            for w in range(NWIN):
                                ge, ct = fetch(layer, w, q)
                                sps = spsum.tile([128, WINB * 128], F32,
                                                 name="sps")
                                for cell in range(WINB):
                                    for j in range(BUDGET):
                                        ch = cell * BUDGET + j
                                        ab = (sps[:, cell * 128:(cell + 1) * 128],
                                              ge[:, ch, :HID], ct[:, ch, :])
                                        if layer >= 2:
                                            ab = (ab[0], ab[2], ab[1])
                                        nc.tensor.matmul(
                                            *ab, start=(j == 0),
                                            stop=(j == BUDGET - 1))
                                asl = acc[:, w * WINB * 128:(w + 1) * WINB * 128]
                                if q == 0:
                                    nc.scalar.activation(asl, sps[:], COPY)
                                else:
                                    nc.vector.tensor_tensor(
                                        out=asl, in0=asl, in1=sps[:],
                                        op=mybir.AluOpType.add)
                                if q == NQ - 1:
                                    if layer == 2:
                                        load_pw(w)
                                    for cell in range(WINB):
                                        b = w * WINB + cell
                                        transform_emit(
                                            layer, b,
                                            acc[:, b * 128:(b + 1) * 128])
                else:
                    for layer in range(3):
                        for w in range(NWIN):
                            pairs = [fetch(layer, w, q) for q in range(NQ)]
                            sps = spsum.tile([128, WINB * 128], F32, name="sps")
                            for cell in range(WINB):
                                for q in range(NQ):
                                    for j in range(BUDGET):
                                        ch = cell * BUDGET + j
                                        ab = (sps[:, cell * 128:(cell + 1) * 128],
                                              pairs[q][0][:, ch, :HID],
                                              pairs[q][1][:, ch, :])
                                        if layer >= 2:
                                            ab = (ab[0], ab[2], ab[1])
                                        nc.tensor.matmul(
                                            *ab, start=(q == 0 and j == 0),
                                            stop=(q == NQ - 1 and j == BUDGET - 1))
                            if layer == 2:
                                load_pw(w)
                            for cell in range(WINB):
                                b = w * WINB + cell
                                transform_emit(
                                    layer, b,
                                    sps[:, cell * 128:(cell + 1) * 128])

                # ---- classifier on pooled partials ----
                gt = eps.tile([128, GSPAN], F16, name="gt", tag="gt")
                nc.vector.tensor_copy(gt[:], gacc[:])
                zps = gpsum.tile([N_CLS, GSPAN], F32, name="zps", tag="zps")
                nc.tensor.matmul(zps[:], wc_t[:], gt[:], start=True, stop=True)
                zsb = eps.tile([N_CLS, GSPAN], F32, name="zsb", tag="zsb")
                nc.vector.tensor_copy(zsb[:], zps[:])
                nc.sync.dma_start(zc[:], zsb[:])

            for _rep in range(reps):
                body(_rep)
    nc.compile()
    return nc


def _in_maps(per_core, shared, loc0=False):
    maps = []
    for c in range(NCORES):
        pc = per_core[c]
        m = {
            "idxs": pc["idxs"], "slots": pc["slots"],
            "slots32": pc["slots32"],
            "iota8": shared["iota8"],
            "W1f": shared["W1f"], "W2f": shared["W2f"], "W3f": shared["W3f"],
            "Wcf": shared["Wcf"],
            "sc12": pc["sc12"], "sc3": pc["sc3"],
            "pstr": pc["pstr"],
        }
        if loc0:
            m["xTf"] = shared["xTf"]
            m["sc1f"] = shared["sc1f"]
        else:
            m["xT"] = pc["xT"]
            m["sc1"] = pc["sc1"]
        maps.append(m)
    return maps


_VARIANT = frozenset({"hag", "shared", "f8", "b4"})


def _prep(variant, inputs):
    budget = 4 if "b4" in variant else BUDGET
    per_core, shared, cnt = _host_prep(
        **inputs, hag=True, budget=budget, bal=("b4" in variant))
    return per_core, shared, cnt, budget


def kernel(**inputs):
    global _COMPILED
    per_core, shared, cnt, budget = _prep(_VARIANT, inputs)
    if _COMPILED is None or _COMPILED[0] != _VARIANT:
        _COMPILED = (_VARIANT, _build_nc(_VARIANT, budget=budget))
    nc = _COMPILED[1]
    res = run_bass_kernel_spmd(nc, _in_maps(per_core, shared, loc0=False),
                               core_ids=list(range(NCORES)))
    Z = np.zeros((N_GRAPHS, N_CLS), np.float64)
    for c in range(NCORES):
        zc_c = res.results[c]["zc"].astype(np.float64)
        g0, span = per_core[c]["g0"], per_core[c]["span"]
        Z[g0:g0 + span] += zc_c[:, :span].T
    Z = Z + np.asarray(inputs["bc"], np.float64)[None, :]
    return Z.astype(np.float32)



# revision 33
# speedup vs baseline: 1.0122x; 1.0122x over previous
"""GNN message-passing (3x GraphConv + mean-pool + classifier) on 8 Trainium2 cores.

v2: quartered, pipelined AllGather (Shared-output) + transposed aggregation.
  - Nodes dst-sharded 8 ways (12500/core, padded to 12800 = 100 blocks of 128).
  - Per-core shard split into 4 quarters of 3200 rows. Each layer's node-major
    fp16 table is published quarter-by-quarter: AllGather(quarter) fires as
    soon as the 25 blocks of that quarter are transformed, overlapping the
    collective with the remaining compute. Gather "pages" = quarters
    (25600 rows each, int16-indexable).
  - Aggregation for layers 0/1 runs feature-major (out = ge^T... via
    matmul(lhsT=ge, rhs=onehot)), so the fused next-layer transform needs no
    PE transpose / DVE copy: matmul(lhsT=ynmT, rhs=W). Layer 2 aggregates
    node-major for the pooling matmul.
  - Per-node norm scales folded into Act-engine activation (Relu / Copy with
    per-partition scale), keeping DVE exclusively for one-hot builds.
"""
import sys
import numpy as np

sys.path.insert(0, "/opt/trn_rl_repo")

import concourse.bass as bass  # noqa: E402
import concourse.bacc as bacc  # noqa: E402
import concourse.mybir as mybir  # noqa: E402
import concourse.tile as tile  # noqa: E402
from concourse.bass_utils import run_bass_kernel_spmd  # noqa: E402

# problem constants
N_NODES = 100000
N_EDGES = 1600000
N_GRAPHS = 1000
IN_DIM, HID, N_CLS = 95, 128, 16

NCORES = 8
B = 12500                        # real nodes per core
BP = 12800                       # padded rows per core (100 blocks)
NBLK = BP // 128                 # 100 blocks
NQ = 4                           # quarters per core
QR = BP // NQ                    # 3200 rows per quarter
QBLK = NBLK // NQ                # 25 blocks per quarter
PAGE = NCORES * QR               # 25600 rows per gather page (one quarter, all cores)
WINB = 4                         # blocks per window
NWIN = NBLK // WINB              # 25 windows
BUDGET = 5                       # chunks (of 128 idx) per (block x page) cell
CELL = BUDGET * 128              # 640 idx slots per cell
CALL = WINB * CELL               # 2560 idxs per dma_gather call
CCOL = CALL // 16                # 160 int16 cols per call in idx stream
NCALL = NWIN * NQ                # 100 calls per layer
CHPC = WINB * BUDGET             # 20 chunks per call
NCHUNK = NCALL * CHPC            # 2000 chunks per layer
GSPAN = 192                      # padded per-core graph span for pooling

F16 = mybir.dt.float16
F32 = mybir.dt.float32
F8 = mybir.dt.float8e4
I16 = mybir.dt.int16

_COMPILED = None


def _dma_gather_128(eng, out_ap, in_ap, idxs_ap, num_idxs, num_idxs_reg,
                    elem_size, elem_step, queue_num):
    """Vendored non-transpose DRAM-source dma_gather with the elem-size
    alignment relaxed from 256B to 128B (the 256B requirement is a
    transpose-path restriction in the ucode decode; non-transpose packets
    carry one descriptor per index of elem_size bytes)."""
    from concourse.bass import MemorySpace
    from concourse import ap_utils
    from concourse._compat import exact_div, round_up_to_multiple

    eng._assert_queue_num(queue_num)
    assert idxs_ap.dtype == mybir.dt.int16
    assert in_ap.dtype == out_ap.dtype
    elem_size_bytes = elem_size * mybir.dt.size(in_ap.dtype)
    assert elem_size_bytes > 0 and elem_size_bytes % 128 == 0
    assert in_ap.space == MemorySpace.DRAM
    assert idxs_ap.space == MemorySpace.SBUF
    assert out_ap.space == MemorySpace.SBUF
    assert ap_utils.ap_is_contiguous(out_ap.ap[1:])
    assert ap_utils.ap_is_contiguous(idxs_ap.ap[1:])
    assert in_ap.ap[-1][1] == out_ap.ap[-1][1] == elem_size
    assert out_ap.ap[0][1] * out_ap.ap[1][1] == round_up_to_multiple(
        num_idxs, 128)
    assert in_ap.ap[0][0] == elem_step
    stride_bytes = elem_step * mybir.dt.size(in_ap.dtype)
    stride_bytes_256 = exact_div(stride_bytes, 256)
    assert stride_bytes_256 < 256
    _in_ap = eng.lower_ap_dma(in_ap, for_custom_bir_dma=True)
    inst = eng.add_instruction(
        mybir.InstDMAGatherAnt(
            name=eng.bass.get_next_instruction_name(),
            ins=[
                *_in_ap,
                eng.lower_ap(idxs_ap),
                eng.lower_val_access(eng.to_reg(num_idxs_reg)),
            ],
            outs=[eng.lower_ap(out_ap)],
            transpose=False,
            num_idxs=num_idxs,
            elem_size=elem_size,
            stride_bytes_256=stride_bytes_256,
            gen_mode=0,
            single_packet=False,
            queue_num=queue_num,
        )
    )
    return inst


HB = B // 2  # real nodes per half under balanced placement (6250)


def _balance_core(prof, cap):
    """Half-constrained greedy placement + repair: local nodes [0,HB) fill
    half-0 positions, [HB,B) fill half-1, permuted so every per-(block,page)
    in-edge count <= cap.

    prof: [B, NQ] per-node in-degree by source page (computed with the same
    half split). Returns pos[B]: table position for each local node."""
    pos = np.zeros(B, np.int64)
    nhb = NBLK // 2
    for h, nodes in ((0, np.arange(0, HB)), (1, np.arange(HB, B))):
        order = nodes[np.argsort(-prof[nodes].max(1), kind="stable")]
        cell = np.zeros((nhb, NQ), np.int64)
        cnt = np.zeros(nhb, np.int64)
        assign = [[] for _ in range(nhb)]
        for v in order:
            proj = np.max(cell + prof[v][None, :], axis=1)
            proj = proj + cnt * 1e-3
            proj[cnt >= 128] = 1 << 30
            b = int(np.argmin(proj))
            assign[b].append(v)
            cell[b] += prof[v]
            cnt[b] += 1
        # repair: relocate heavy contributors out of over-cap cells
        for _ in range(20000):
            bs, ps = np.unravel_index(np.argmax(cell), cell.shape)
            worst = cell[bs, ps]
            if worst <= cap:
                break
            best = None
            for v in sorted(assign[bs], key=lambda u: -prof[u][ps])[:12]:
                if prof[v][ps] == 0:
                    break
                proj = np.max(cell + prof[v][None, :], axis=1)
                proj[cnt >= 128] = 1 << 30
                proj[bs] = 1 << 30
                b2 = int(np.argmin(proj))
                if proj[b2] < worst:
                    best = (v, b2)
                    break
            if best is None:
                break
            v, b2 = best
            assign[bs].remove(v)
            assign[b2].append(v)
            cell[bs] -= prof[v]
            cell[b2] += prof[v]
            cnt[bs] -= 1
            cnt[b2] += 1
        for b in range(nhb):
            for i, v in enumerate(assign[b]):
                pos[v] = (h * nhb + b) * 128 + i
    return pos


def _host_prep(x, src, dst, graph_id, W1, b1, W2, b2, W3, b3, Wc, bc,
               fullag=False, hag=False, budget=BUDGET, bal=False):
    """Build all per-core input streams. Index math only (+ dtype marshaling)."""
    CELL = budget * 128
    CALL = WINB * CELL
    CCOL = CALL // 16
    CHPC = WINB * budget
    src = np.asarray(src).astype(np.int64)
    dst = np.asarray(dst).astype(np.int64)
    graph_id = np.asarray(graph_id).astype(np.int64)
    x = np.asarray(x, dtype=np.float32)
    assert np.all(np.asarray(b1) == 0) and np.all(np.asarray(b2) == 0) and np.all(
        np.asarray(b3) == 0
    ), "kernel assumes zero conv biases (relu/scale folding)"

    deg_out = np.clip(np.bincount(src, minlength=N_NODES), 1, None).astype(np.float64)
    deg_in = np.clip(np.bincount(dst, minlength=N_NODES), 1, None).astype(np.float64)
    ns = (deg_out ** -0.5).astype(np.float32)
    nd = (deg_in ** -0.5).astype(np.float32)
    cnt = np.clip(np.bincount(graph_id, minlength=N_GRAPHS), 1, None).astype(np.float32)

    s_oc = src // B
    s_r = src % B
    # page of a source depends only on its quarter/half, which balancing
    # preserves, so compute pages from the identity layout first
    if fullag:
        s_page = s_oc // 2
    elif hag:
        s_page = (s_r // (2 * QR)) * 2 + s_oc // 4
    else:
        s_page = s_r // QR

    if bal:
        assert hag, "balanced placement only wired for hag page layout"
        # planned half split (HB real nodes per half) fixes every source's
        # page up front; balancing then permutes within halves only
        s_page_b = ((s_r >= HB).astype(np.int64)) * 2 + s_oc // 4
        pos_all = np.zeros(N_NODES, np.int64)
        prof_all = np.zeros((N_NODES, NQ), np.int64)
        np.add.at(prof_all, (dst, s_page_b), 1)
        for c in range(NCORES):
            gl = slice(c * B, (c + 1) * B)
            pos_all[gl] = _balance_core(prof_all[gl], CELL) + c * BP
    else:
        ids = np.arange(N_NODES, dtype=np.int64)
        pos_all = (ids // B) * BP + (ids % B)

    p_src = pos_all[src] % BP  # per-edge: source position within owner core
    if fullag:
        s_row = (s_oc % 2) * BP + p_src
    elif hag:
        s_h = p_src // (2 * QR)
        s_row = (s_oc % 4) * (2 * QR) + (p_src - s_h * 2 * QR)
        s_page = s_h * 2 + s_oc // 4
    else:
        s_row = s_oc * QR + (p_src % QR)

    core_of = dst // B
    per_core = []
    for c in range(NCORES):
        m = core_of == c
        ed = pos_all[dst[m]] - c * BP
        blk = ed >> 7
        page = s_page[m]
        lrow = s_row[m]
        slot = (ed & 127).astype(np.int64)

        idx_stream = np.zeros((NCALL, CALL), np.int64)
        slot_stream = np.full((NCALL, CALL), -1.0, np.float32)

        # bucket edges by (block, page); sort by gather row for locality
        order = np.lexsort((lrow, page, blk))
        blk_o, page_o, lrow_o, slot_o = (
            blk[order], page[order], lrow[order], slot[order])
        cell_key = blk_o * NQ + page_o
        starts = np.searchsorted(cell_key, np.arange(NBLK * NQ + 1))
        for b in range(NBLK):
            w, cw_ = divmod(b, WINB)
            for q in range(NQ):
                lo, hi = starts[b * NQ + q], starts[b * NQ + q + 1]
                n = hi - lo
                assert n <= CELL, f"cell overflow core {c} blk {b} page {q}: {n}"
                call_id = w * NQ + q
                base = cw_ * CELL
                idx_stream[call_id, base:base + n] = lrow_o[lo:hi]
                slot_stream[call_id, base:base + n] = slot_o[lo:hi]
                if n < CELL:  # mid-call pads: repeat a valid row (slot=-1)
                    fill = lrow_o[hi - 1] if n > 0 else 0
                    idx_stream[call_id, base + n:base + CELL] = fill

        # wrap idxs: flat position i -> [i%16, i//16], replicated to 128 partitions
        idx16 = idx_stream.reshape(NCALL, CCOL, 16).transpose(0, 2, 1)
        idx128 = np.tile(idx16, (1, 8, 1)).transpose(1, 0, 2).reshape(128, NCALL * CCOL)
        slot128 = slot_stream.reshape(NCALL * CHPC, 128).T.copy()

        # node -> position scatter for this core
        pos_c = pos_all[c * B:(c + 1) * B] - c * BP

        # pooling stream [128, NBLK*GSPAN]; padded rows contribute zero
        gid_c = graph_id[c * B:(c + 1) * B]
        g0 = int(gid_c.min())
        span = int(gid_c.max()) - g0 + 1
        assert span <= GSPAN, f"graph span {span} > {GSPAN}"
        pstream = np.zeros((128, NBLK * GSPAN), np.float32)
        pstream[pos_c & 127, (pos_c >> 7) * GSPAN + (gid_c - g0)] = 1.0 / cnt[gid_c]

        # per-node scalar streams [128, NBLK], zero in padded region
        def colify(v):
            flat = np.zeros(BP, np.float32)
            flat[pos_c] = v[c * B:(c + 1) * B]
            return np.ascontiguousarray(flat.reshape(NBLK, 128).T)

        xp = np.zeros((BP, IN_DIM), np.float32)
        xp[pos_c] = x[c * B:(c + 1) * B]
        xT = np.ascontiguousarray(xp.T).astype(np.float16)  # [95, BP]

        per_core.append({
            "xT": xT,
            "idxs": idx128.astype(np.int16),
            "slots": slot128.astype(np.float16),
            "slots32": slot128.astype(np.float32),
            "sc1": colify(ns),
            "sc12": colify(ns * nd),
            "sc3": colify(nd),
            "pstr": pstream.astype(np.float16),
            "g0": g0, "span": span,
        })

    # loc0 streams: full padded x^T and full ns (block-column layout), same on
    # every core
    xpf = np.zeros((NCORES * BP, IN_DIM), np.float32)
    nsf = np.zeros(NCORES * BP, np.float32)
    for c in range(NCORES):
        xpf[c * BP:c * BP + B] = x[c * B:(c + 1) * B]
        nsf[c * BP:c * BP + B] = ns[c * B:(c + 1) * B]
    xTf = np.ascontiguousarray(xpf.T).astype(np.float16)  # [95, 8*BP]
    sc1f = np.ascontiguousarray(nsf.reshape(NCORES * NBLK, 128).T).astype(np.float32)

    iota = np.ascontiguousarray(np.broadcast_to(
        np.arange(128, dtype=np.float16)[None, None, :], (128, CHPC, 128)))
    shared = {
        "xTf": xTf,
        "sc1f": sc1f,
        "iota8": iota.reshape(128, CHPC * 128),
        "W1f": np.asarray(W1, np.float32).astype(np.float16),
        "W2f": np.asarray(W2, np.float32).astype(np.float16),
        "W3f": np.asarray(W3, np.float32).astype(np.float16),
        "Wcf": np.asarray(Wc, np.float32).astype(np.float16),
    }
    return per_core, shared, cnt


def _build_nc(variant=frozenset(), reps=1, budget=BUDGET):
    variant = frozenset(variant)
    CELL = budget * 128
    CALL = WINB * CELL
    CCOL = CALL // 16
    CHPC = WINB * budget
    NCHUNK = NCALL * CHPC
    BUDGET = budget  # noqa: F841 — shadow module constant for loops below
    nqueues = 8 if "q8" in variant else 4
    nc = bacc.Bacc("TRN2", target_bir_lowering=False, debug=False,
                   num_devices=NCORES, num_swdge_queues=nqueues)
    loc0 = "loc0" in variant
    if loc0:
        xTf = nc.dram_tensor("xTf", [IN_DIM, NCORES * BP], F16, kind="ExternalInput")
        sc1f = nc.dram_tensor("sc1f", [128, NCORES * NBLK], F32, kind="ExternalInput")
    else:
        xT = nc.dram_tensor("xT", [IN_DIM, BP], F16, kind="ExternalInput")
    idxs = nc.dram_tensor("idxs", [128, NCALL * CCOL], I16, kind="ExternalInput")
    slots = None if "tshot" in variant else nc.dram_tensor(
        "slots", [128, NCHUNK], F16, kind="ExternalInput")
    slots32 = nc.dram_tensor("slots32", [128, NCHUNK], F32, kind="ExternalInput") \
        if "tshot" in variant else None
    iota8 = nc.dram_tensor("iota8", [128, CHPC * 128], F16, kind="ExternalInput")
    W1f = nc.dram_tensor("W1f", [IN_DIM, HID], F16, kind="ExternalInput")
    W2f = nc.dram_tensor("W2f", [HID, HID], F16, kind="ExternalInput")
    W3f = nc.dram_tensor("W3f", [HID, HID], F16, kind="ExternalInput")
    Wcf = nc.dram_tensor("Wcf", [HID, N_CLS], F16, kind="ExternalInput")
    if not loc0:
        sc1 = nc.dram_tensor("sc1", [128, NBLK], F32, kind="ExternalInput")
    sc12 = nc.dram_tensor("sc12", [128, NBLK], F32, kind="ExternalInput")
    sc3 = nc.dram_tensor("sc3", [128, NBLK], F32, kind="ExternalInput")
    pstr = nc.dram_tensor("pstr", [128, NBLK * GSPAN], F16, kind="ExternalInput")
    zc = nc.dram_tensor("zc", [N_CLS, GSPAN], F32, kind="ExternalOutput")

    RELU = mybir.ActivationFunctionType.Relu
    COPY = mybir.ActivationFunctionType.Copy

    with tile.TileContext(nc) as tc:
        with (
            tc.tile_pool(name="res", bufs=1) as res,
            tc.tile_pool(name="dram", bufs=1, space="DRAM") as dram,
            tc.tile_pool(name="gew", bufs=(6 if "fat" in variant else
                                           12 if "cw14" not in variant else 10)) as gew,
            tc.tile_pool(name="cw", bufs=(15 if "cwx" in variant else
                                          12 if "cw14" not in variant else 14)) as cw,
            tc.tile_pool(name="eps", bufs=6) as eps,
            tc.tile_pool(name="accp", bufs=1) as accp,
            tc.tile_pool(name="pwin", bufs=2) as pwin,
            tc.tile_pool(name="xsp", bufs=3) as xsp,
            tc.tile_pool(name="spsum", bufs=2, space="PSUM") as spsum,
            tc.tile_pool(name="tpsum", bufs=2, space="PSUM") as tpsum,
            tc.tile_pool(name="gpsum", bufs=1, space="PSUM") as gpsum,
        ):
            # resident loads
            idx_t = res.tile([128, NCALL * CCOL], I16)
            slot_t = None if "tshot" in variant else res.tile(
                [128, NCHUNK], F16, name="slot_t")
            slot32_t = res.tile([128, NCHUNK], F32, name="slot32_t") \
                if "tshot" in variant else None
            iota_t = res.tile([128, CHPC, 128], F16)
            xT_t = None if (loc0 or "cwx" in variant) else res.tile(
                [IN_DIM, BP], F16, name="xT_t")
            w1_t = res.tile([IN_DIM, HID], F16)
            w2_t = res.tile([HID, HID], F16)
            w3_t = res.tile([HID, HID], F16)
            wc_t = res.tile([HID, N_CLS], F16)
            sc1_t = None if loc0 else res.tile([128, NBLK], F32, name="sc1_t")
            sc1f_t = res.tile([128, NCORES * NBLK], F32, name="sc1f_t") if loc0 else None
            sc12_t = res.tile([128, NBLK], F32)
            sc3_t = res.tile([128, NBLK], F32)
            nc.sync.dma_start(idx_t[:], idxs[:])
            if slot_t is not None:
                nc.sync.dma_start(slot_t[:], slots[:])
            if slot32_t is not None:
                nc.sync.dma_start(slot32_t[:], slots32[:])
            nc.sync.dma_start(iota_t[:], iota8[:].rearrange("p (a b) -> p a b", a=CHPC))
            if loc0:
                nc.sync.dma_start(sc1f_t[:], sc1f[:])
            elif xT_t is not None:
                nc.sync.dma_start(xT_t[:], xT[:])
            nc.sync.dma_start(w1_t[:], W1f[:])
            nc.sync.dma_start(w2_t[:], W2f[:])
            nc.sync.dma_start(w3_t[:], W3f[:])
            nc.sync.dma_start(wc_t[:], Wcf[:])
            if not loc0:
                nc.sync.dma_start(sc1_t[:], sc1[:])
            nc.sync.dma_start(sc12_t[:], sc12[:])
            nc.sync.dma_start(sc3_t[:], sc3[:])

            yaddr = "Shared" if "shared" in variant else "Local"
            ybq = [[dram.tile([QR, HID], F16, name=f"yb{i}_{q}", tag=f"yb{i}_{q}")
                    for q in range(NQ)] for i in range(3)]

            def publish(yfp, layer, q):
                if "nocc" in variant:
                    return
                nc.gpsimd.collective_compute(
                    "AllGather", mybir.AluOpType.bypass,
                    replica_groups=[list(range(NCORES))],
                    ins=[ybq[layer][q][:].opt()], outs=[yfp[layer][q].opt()])

            def body(rep):
                ctd = (dram.tile([128, NCHUNK * 128], F16, name=f"ctd{rep}",
                                 tag=f"ctd{rep}")
                       if "ctcache" in variant else None)
                ctd2 = ([dram.tile([128, CHPC * 128], F16, name=f"ctd2_{rep}_{i}",
                                   tag=f"ctd2_{rep}_{i}") for i in range(NCALL)]
                        if "ctc2" in variant else None)
                f8 = "f8" in variant
                tdt = F8 if f8 else F16
                tw = 2 * HID if f8 else HID  # f8 rows padded to 256B stride
                if "hag" in variant:
                    ybh = [[dram.tile([2 * QR, tw], tdt, name=f"ybh{rep}_{i}_{h}",
                                      tag=f"ybh{rep}_{i}_{h}") for h in range(2)]
                           for i in range(3)]
                    yfh = [[dram.tile([2 * PAGE, tw], tdt, name=f"yfh{rep}_{i}_{h}",
                                      tag=f"yfh{rep}_{i}_{h}", addr_space=yaddr)
                            for h in range(2)] for i in range(3)]
                    yfp = [[yfh[i][q // 2][(q % 2) * PAGE:(q % 2 + 1) * PAGE, :]
                            for q in range(NQ)] for i in range(3)]
                elif "fullag" in variant:
                    ybf = [dram.tile([BP, HID], F16, name=f"ybf{rep}_{i}",
                                     tag=f"ybf{rep}_{i}") for i in range(3)]
                    yff = [dram.tile([NCORES * BP, HID], F16, name=f"yff{rep}_{i}",
                                     tag=f"yff{rep}_{i}", addr_space=yaddr)
                        for i in range(3)]
                    yfp = [[yff[i][q * PAGE:(q + 1) * PAGE, :] for q in range(NQ)]
                           for i in range(3)]
                else:
                    yfp = [[dram.tile([PAGE, HID], F16, name=f"yf{rep}_{i}_{q}",
                                      tag=f"yf{rep}_{i}_{q}",
                                      addr_space=("Local" if (loc0 and i == 0)
                                                  else yaddr))[:]
                            for q in range(NQ)] for i in range(3)]

                def emit_table_row(layer, b, tnm):
                    if "hag" in variant:
                        hh, bh = divmod(b, 2 * QBLK)
                        nc.sync.dma_start(
                            ybh[layer][hh][bh * 128:(bh + 1) * 128, :HID], tnm[:])
                        if bh == 2 * QBLK - 1 and "nocc" not in variant:
                            nc.gpsimd.collective_compute(
                                "AllGather", mybir.AluOpType.bypass,
                                replica_groups=[list(range(NCORES))],
                                ins=[ybh[layer][hh][:].opt()],
                                outs=[yfh[layer][hh][:].opt()])
                        return
                    if "fullag" in variant:
                        nc.sync.dma_start(ybf[layer][b * 128:(b + 1) * 128, :], tnm[:])
                        if b == NBLK - 1 and "nocc" not in variant:
                            nc.gpsimd.collective_compute(
                                "AllGather", mybir.AluOpType.bypass,
                                replica_groups=[list(range(NCORES))],
                                ins=[ybf[layer][:].opt()], outs=[yff[layer][:].opt()])
                    else:
                        qq, bq = divmod(b, QBLK)
                        nc.sync.dma_start(
                            ybq[layer][qq][bq * 128:(bq + 1) * 128, :], tnm[:])
                        if bq == QBLK - 1:
                            publish(yfp, layer, qq)
                # ---- phase T1: table0 = (x @ W1) * ns ----
                if loc0:
                    # every core computes the FULL table0 from replicated x:
                    # no layer-0 collectives; gathers then read local HBM
                    for gbase in range(0, NCORES * NBLK, 8):
                        xs = xsp.tile([IN_DIM, 8, 128], F16, name="xs", tag="xs")
                        nc.sync.dma_start(
                            xs[:], xTf[:, gbase * 128:(gbase + 8) * 128]
                            .rearrange("p (a b) -> p a b", a=8))
                        for k in range(8):
                            gb = gbase + k
                            oc, b = divmod(gb, NBLK)
                            qq, bq = divmod(b, QBLK)
                            tps = tpsum.tile([128, HID], F32, name="tps")
                            nc.tensor.matmul(tps[:], xs[:, k, :], w1_t[:],
                                             start=True, stop=True)
                            tnm = eps.tile([128, HID], tdt, name="tnm", tag="tnm")
                            nc.scalar.activation(tnm[:], tps[:], COPY,
                                                 scale=sc1f_t[:, gb:gb + 1])
                            row = oc * QR + bq * 128
                            nc.sync.dma_start(yfp[0][qq][row:row + 128, :], tnm[:])
                elif "cwx" in variant:
                    for bbase in range(0, NBLK, 4):
                        xs = xsp.tile([IN_DIM, 4, 128], F16, name="xs", tag="xs")
                        nc.sync.dma_start(
                            xs[:], xT[:, bbase * 128:(bbase + 4) * 128]
                            .rearrange("p (a b) -> p a b", a=4))
                        for k in range(4):
                            b = bbase + k
                            tps = tpsum.tile([128, HID], F32, name="tps")
                            nc.tensor.matmul(tps[:], xs[:, k, :], w1_t[:],
                                             start=True, stop=True)
                            tnm = eps.tile([128, HID], tdt, name="tnm", tag="tnm")
                            nc.scalar.activation(tnm[:], tps[:], COPY,
                                                 scale=sc1_t[:, b:b + 1])
                            emit_table_row(0, b, tnm)
                else:
                    for b in range(NBLK):
                        tps = tpsum.tile([128, HID], F32, name="tps")
                        nc.tensor.matmul(tps[:], xT_t[:, b * 128:(b + 1) * 128],
                                         w1_t[:], start=True, stop=True)
                        tnm = eps.tile([128, HID], tdt, name="tnm", tag="tnm")
                        nc.scalar.activation(tnm[:], tps[:], COPY,
                                             scale=sc1_t[:, b:b + 1])
                        emit_table_row(0, b, tnm)

                gacc = gpsum.tile([128, GSPAN], F32, name="gacc")

                def fetch(layer, w, q):
                    """gather + one-hot build for call (w, q)."""
                    call_id = w * NQ + q
                    qno = call_id % 8 if "q8" in variant else q
                    fat = "fat" in variant
                    ge = gew.tile([128, CHPC, 2 * HID if fat else HID],
                                  tdt, name="ge", tag="ge")
                    if "nogather" in variant:
                        nc.sync.dma_start(
                            ge[:], yfp[layer][q][:128 * CHPC, :HID]
                            .rearrange("(p a) h -> p a h", p=128))
                    elif f8:
                        _dma_gather_128(
                            nc.gpsimd, out_ap=ge[:],
                            in_ap=yfp[layer][q][:, :HID],
                            idxs_ap=idx_t[:, call_id * CCOL:(call_id + 1) * CCOL],
                            num_idxs=CALL, num_idxs_reg=CALL,
                            elem_size=HID, elem_step=2 * HID,
                            queue_num=qno)
                    else:
                        in_ap = (yfp[layer][q]
                                 .rearrange("(a b) h -> a (b h)", b=2)
                                 if fat else yfp[layer][q])
                        nc.gpsimd.dma_gather(
                            out_ap=ge[:],
                            in_ap=in_ap,
                            idxs_ap=idx_t[:, call_id * CCOL:(call_id + 1) * CCOL],
                            num_idxs=CALL, num_idxs_reg=CALL,
                            elem_size=2 * HID if fat else HID,
                            single_packet=("sp" in variant), queue_num=qno)
                    ct = cw.tile([128, CHPC, 128],
                                 F8 if "ct8" in variant else F16,
                                 name="ct", tag="ct")
                    cbase = call_id * CHPC
                    if "noonehot" in variant:
                        nc.vector.tensor_copy(ct[:], iota_t[:])
                    elif "tshot" in variant:
                        for ch in range(CHPC):
                            nc.vector.tensor_scalar(
                                out=ct[:, ch, :], in0=iota_t[:, 0, :],
                                scalar1=slot32_t[:, cbase + ch:cbase + ch + 1],
                                scalar2=None,
                                op0=mybir.AluOpType.is_equal)
                    else:
                        nc.vector.tensor_tensor(
                            out=ct[:],
                            in0=slot_t[:, cbase:cbase + CHPC, None]
                                .to_broadcast([128, CHPC, 128]),
                            in1=iota_t[:],
                            op=mybir.AluOpType.is_equal)
                    return ge, ct

                def transform_emit(layer, b, src_ap):
                    """L0/L1: relu -> W-transform -> scaled emit; L2: pool."""
                    wnext = [w2_t, w3_t, None][layer]
                    scale_t = sc12_t if layer < 2 else sc3_t
                    if layer < 2:
                        ynmT = eps.tile([128, HID], F16, name="ynmT", tag="ynmT")
                        nc.scalar.activation(ynmT[:], src_ap, RELU)
                        tps2 = tpsum.tile([128, HID], F32, name="tps2")
                        nc.tensor.matmul(tps2[:], ynmT[:], wnext[:],
                                         start=True, stop=True)
                        tnm = eps.tile([128, HID], tdt, name="tnm", tag="tnm")
                        nc.scalar.activation(tnm[:], tps2[:], COPY,
                                             scale=scale_t[:, b:b + 1])
                        emit_table_row(layer + 1, b, tnm)
                    else:
                        ynm = eps.tile([128, HID], F16, name="ynm", tag="ynm")
                        nc.scalar.activation(ynm[:], src_ap, RELU,
                                             scale=scale_t[:, b:b + 1])
                        nc.tensor.matmul(
                            gacc[:], ynm[:], pw_t[b // WINB][:, b % WINB, :],
                            start=(b == 0), stop=(b == NBLK - 1))

                pw_t = {}

                def load_pw(w):
                    pw = pwin.tile([128, WINB, GSPAN], F16, name="pw")
                    nc.sync.dma_start(
                        pw[:],
                        pstr[:, w * WINB * GSPAN:(w + 1) * WINB * GSPAN]
                        .rearrange("p (a g) -> p a g", a=WINB))
                    pw_t[w] = pw

                # ---- layers ----
                if "pm" in variant:
                    # page-major: sweep pages in publish order, accumulate in
                    # an SBUF fp32 table; collectives hide under the stream
                    for layer in range(3):
                        wacc = [None] * NWIN
                        for q in range(NQ):
                            for w in range(NWIN):
                                ge, ct = fetch(layer, w, q)
                                sps = spsum.tile([128, WINB * 128], F32,
                                                 name="sps")
                                for cell in range(WINB):
                                    for j in range(BUDGET):
                                        ch = cell * BUDGET + j
                                        ab = (sps[:, cell * 128:(cell + 1) * 128],
                                              ge[:, ch, :HID], ct[:, ch, :])
                                        if layer >= 2:
                                            ab = (ab[0], ab[2], ab[1])
                                        nc.tensor.matmul(
                                            *ab, start=(j == 0),
                                            stop=(j == BUDGET - 1))
                                if q == 0:
                                    wacc[w] = accp.tile(
                                        [128, WINB * 128], F16,
                                        name=f"acc{w}", tag=f"acc{w}")
                                    nc.scalar.activation(wacc[w][:], sps[:], COPY)
                                else:
                                    nc.vector.tensor_tensor(
                                        out=wacc[w][:], in0=wacc[w][:],
                                        in1=sps[:], op=mybir.AluOpType.add)
                                if q == NQ - 1:
                                    if layer == 2:
                                        load_pw(w)
                                    for cell in range(WINB):
                                        b = w * WINB + cell
                                        transform_emit(
                                            layer, b,
                                            wacc[w][:, cell * 128:(cell + 1) * 128])
                else:
                    for layer in range(3):
                        for w in range(NWIN):
                            pairs = [fetch(layer, w, q) for q in range(NQ)]
                            sps = spsum.tile([128, WINB * 128], F32, name="sps")
                            for cell in range(WINB):
                                for q in range(NQ):
                                    for j in range(BUDGET):
                                        ch = cell * BUDGET + j
                                        ab = (sps[:, cell * 128:(cell + 1) * 128],
                                              pairs[q][0][:, ch, :HID],
                                              pairs[q][1][:, ch, :])
                                        if layer >= 2:
                                            ab = (ab[0], ab[2], ab[1])
                                        nc.tensor.matmul(
                                            *ab, start=(q == 0 and j == 0),
                                            stop=(q == NQ - 1 and j == BUDGET - 1))
                            if layer == 2:
                                load_pw(w)
                            for cell in range(WINB):
                                b = w * WINB + cell
                                transform_emit(
                                    layer, b,
                                    sps[:, cell * 128:(cell + 1) * 128])

                # ---- classifier on pooled partials ----
                gt = eps.tile([128, GSPAN], F16, name="gt", tag="gt")
                nc.vector.tensor_copy(gt[:], gacc[:])
                zps = gpsum.tile([N_CLS, GSPAN], F32, name="zps", tag="zps")
                nc.tensor.matmul(zps[:], wc_t[:], gt[:], start=True, stop=True)
                zsb = eps.tile([N_CLS, GSPAN], F32, name="zsb", tag="zsb")
                nc.vector.tensor_copy(zsb[:], zps[:])
                nc.sync.dma_start(zc[:], zsb[:])

            for _rep in range(reps):
                body(_rep)
    nc.compile()
    return nc


def _in_maps(per_core, shared, loc0=False):
    maps = []
    for c in range(NCORES):
        pc = per_core[c]
        m = {
            "idxs": pc["idxs"], "slots": pc["slots"],
            "slots32": pc["slots32"],
            "iota8": shared["iota8"],
            "W1f": shared["W1f"], "W2f": shared["W2f"], "W3f": shared["W3f"],
            "Wcf": shared["Wcf"],
            "sc12": pc["sc12"], "sc3": pc["sc3"],
            "pstr": pc["pstr"],
        }
        if loc0:
            m["xTf"] = shared["xTf"]
            m["sc1f"] = shared["sc1f"]
        else:
            m["xT"] = pc["xT"]
            m["sc1"] = pc["sc1"]
        maps.append(m)
    return maps


_VARIANT = frozenset({"hag", "shared", "f8", "b4"})


def _prep(variant, inputs):
    budget = 4 if "b4" in variant else BUDGET
    per_core, shared, cnt = _host_prep(
        **inputs, hag=True, budget=budget, bal=("b4" in variant))
    return per_core, shared, cnt, budget


def kernel(**inputs):
    global _COMPILED
    per_core, shared, cnt, budget = _prep(_VARIANT, inputs)
    if _COMPILED is None or _COMPILED[0] != _VARIANT:
        _COMPILED = (_VARIANT, _build_nc(_VARIANT, budget=budget))
    nc = _COMPILED[1]
    res = run_bass_kernel_spmd(nc, _in_maps(per_core, shared, loc0=False),
                               core_ids=list(range(NCORES)))
    Z = np.zeros((N_GRAPHS, N_CLS), np.float64)
    for c in range(NCORES):
        zc_c = res.results[c]["zc"].astype(np.float64)
        g0, span = per_core[c]["g0"], per_core[c]["span"]
        Z[g0:g0 + span] += zc_c[:, :span].T
    Z = Z + np.asarray(inputs["bc"], np.float64)[None, :]
    return Z.astype(np.float32)

